# revision 7
# baseline (speedup 1.0000x reference)
"""Trainium2 Bass kernel for the MemoryANN recurrence.

Model (per batch row b, per step t over T-1 steps):
    sr_t = tanh(w_r1 @ [r_t; sr_{t-1}] + b_r1)          # 32-dim reward module
    qn_t = w_r2 @ sr_t + b_r2                           # scalar q_new
    sa_t = tanh(w_a1 @ [oh_t; sa_{t-1}] + b_a1)         # 32-dim action module
    c_t  = w_a2 @ sa_t + b_a2                           # 4-dim
    q_t  = 0.95*q_{t-1}*(1-oh_t) + qn_t*oh_t            # gated decay
    logits_t = q_t + c_t

Device strategy (pure data parallel, 1024 batch rows per core):
  Phase 1 (serial over t): feature-major state X [70, 1024] bf16 holds
  [sr(32); sa(32); ones(1); r(1); oh(4)].  One fused matmul produces both
  pre-tanh vectors (PSUM fp32), one ACT instr applies tanh+bias, a second
  fused matmul produces [c_t(4); qn_t(x4)] which is copied to SBUF and
  DMA'd to a DRAM scratch, feature-major.
  Phase 2 (batch-parallel): the q recurrence is a per-(b,j) linear scan
  q = A*q + Bv with A = 0.95*(1-oh), Bv = oh*qn.  Transpose qn/c to
  batch-major with PE transposes, then DVE tensor_tensor_scan runs the
  whole T recurrence in one instruction per (chunk, action).
"""

import sys

if "/root/.axon_site/_ro/trn_rl_repo" not in sys.path:  # pragma: no cover
    sys.path.insert(0, "/root/.axon_site/_ro/trn_rl_repo")

from contextlib import ExitStack

import numpy as np
import ml_dtypes

import concourse.bass as bass
import concourse.bacc as bacc
import concourse.tile as tile
from concourse import mybir
from concourse.bass_utils import run_bass_kernel_spmd

BF16 = mybir.dt.bfloat16
F32 = mybir.dt.float32
F32R = mybir.dt.float32r

import os
B, T = 8192, 1024
TS = int(os.environ.get("BASS_TS", T - 1))  # recurrence steps (default 1023)
NCORES = 8
B_LOC = B // NCORES  # 1024
N_ACT = 4
FORGET = 0.05
DECAY = 1.0 - FORGET

_CACHE = {}


def _build_nc():
    nc = bacc.Bacc("TRN2", target_bir_lowering=False, debug=False,
                   num_devices=NCORES)

    io_d = nc.dram_tensor("io", [TS, 5, B_LOC], F32, kind="ExternalInput")
    w1_d = nc.dram_tensor("w1", [70, 64], F32, kind="ExternalInput")
    w2_d = nc.dram_tensor("w2", [65, 8], F32, kind="ExternalInput")
    bias_d = nc.dram_tensor("bias", [64, 1], F32, kind="ExternalInput")
    abm_d = nc.dram_tensor("abm", [8, 128, TS], F32, kind="ExternalInput")
    eye_d = nc.dram_tensor("eye", [128, 128], F32, kind="ExternalInput")
    init_d = nc.dram_tensor("init", [65, B_LOC], F32, kind="ExternalInput")
    out2_d = nc.dram_tensor("out2", [1024, 8, B_LOC], F32)  # internal scratch
    lg_d = nc.dram_tensor("lg", [8, N_ACT, 128, TS], F32, kind="ExternalOutput")

    with tile.TileContext(nc) as tc:
        _kernel(tc, io_d, w1_d, w2_d, bias_d, abm_d, eye_d, init_d, out2_d, lg_d)
    nc.compile()
    return nc


def _kernel(tc, io_d, w1_d, w2_d, bias_d, abm_d, eye_d, init_d, out2_d, lg_d):
    nc = tc.nc
    with ExitStack() as ctx:
        const = ctx.enter_context(tc.tile_pool(name="const", bufs=1))
        w1 = const.tile([70, 64], F32R, tag="w1")
        w2 = const.tile([65, 8], F32R, tag="w2")
        bias = const.tile([64, 1], F32, tag="bias")
        eye = const.tile([128, 128], F32, tag="eye")
        nc.sync.dma_start(w1[:], w1_d[:].bitcast(F32R))
        nc.sync.dma_start(w2[:], w2_d[:].bitcast(F32R))
        nc.sync.dma_start(bias[:], bias_d[:])
        nc.sync.dma_start(eye[:], eye_d[:])

        # ---------------- Phase 1: the serial recurrence ----------------
        with ExitStack() as p1ctx:
            xpool = p1ctx.enter_context(tc.tile_pool(name="xs", bufs=1))
            ps1 = p1ctx.enter_context(
                tc.tile_pool(name="ps1", bufs=2, space="PSUM"))
            ps2 = p1ctx.enter_context(
                tc.tile_pool(name="ps2", bufs=2, space="PSUM"))
            stag = p1ctx.enter_context(tc.tile_pool(name="stag", bufs=4))

            NX = 4  # ping-pong depth for the state/io buffer
            xs = [xpool.tile([70, B_LOC], F32R, tag=f"x{i}", name=f"x{i}")
                  for i in range(NX)]
            # init: zeros state for x0, ones row (bias multiplier) for all
            nc.sync.dma_start(xs[0][0:65, :], init_d[:].bitcast(F32R))
            for x in xs[1:]:
                nc.sync.dma_start(x[64:65, :], init_d[64:65, :].bitcast(F32R))

            for t in range(TS):
                xc = xs[t % NX]
                xn = xs[(t + 1) % NX]
                nc.sync.dma_start(xc[65:70, :], io_d[t].bitcast(F32R))

                p1 = ps1.tile([64, B_LOC], F32, tag="p1")
                nc.tensor.matmul(p1[:, 0:512], w1[:], xc[:, 0:512])
                nc.tensor.matmul(p1[:, 512:1024], w1[:], xc[:, 512:1024])
                nc.scalar.activation(xn[0:64, :], p1[:, :],
                                     mybir.ActivationFunctionType.Tanh,
                                     bias=bias[:, 0:1])

                p2 = ps2.tile([8, B_LOC], F32, tag="p2")
                nc.tensor.matmul(p2[:, 0:512], w2[:], xn[0:65, 0:512])
                nc.tensor.matmul(p2[:, 512:1024], w2[:], xn[0:65, 512:1024])
                st = stag.tile([8, B_LOC], F32, tag="st")
                nc.vector.tensor_copy(st[:], p2[:])
                nc.sync.dma_start(out2_d[t], st[:])

        # ---------------- Phase 2: batched q-scan ----------------
        with ExitStack() as p2ctx:
            pool = p2ctx.enter_context(tc.tile_pool(name="ph2", bufs=2))
            big = p2ctx.enter_context(tc.tile_pool(name="ph2big", bufs=2))
            pst = p2ctx.enter_context(
                tc.tile_pool(name="pst", bufs=2, space="PSUM"))

            for c in range(8):  # batch chunks of 128
                abm = big.tile([128, TS], F32, tag="abm")
                nc.sync.dma_start(abm[:], abm_d[c])
                qnc = big.tile([128, 5, 1024], F32, tag="qnc")
                for g in range(8):  # time groups of 128
                    fm = pool.tile([128, 5, 128], F32, tag="fm")
                    nc.sync.dma_start(
                        fm[:],
                        out2_d[g * 128:(g + 1) * 128, 0:5,
                               c * 128:(c + 1) * 128])
                    pt = pst.tile([128, 5, 128], F32, tag="pt")
                    for f in range(5):
                        nc.tensor.transpose(pt[:, f, :], fm[:, f, :], eye[:])
                    nc.vector.tensor_copy(
                        qnc[:, :, g * 128:(g + 1) * 128], pt[:])

                for j in range(N_ACT):
                    oh = big.tile([128, TS], F32, tag="oh")
                    nc.vector.tensor_scalar(oh[:], abm[:], float(j), None,
                                            mybir.AluOpType.is_equal)
                    av = big.tile([128, TS], F32, tag="av")
                    nc.vector.tensor_scalar(av[:], oh[:], -DECAY, DECAY,
                                            mybir.AluOpType.mult,
                                            mybir.AluOpType.add)
                    bv = big.tile([128, TS], F32, tag="bv")
                    nc.vector.tensor_tensor(bv[:], oh[:], qnc[:, 4, 0:TS],
                                            mybir.AluOpType.mult)
                    q = big.tile([128, TS], F32, tag="q")
                    nc.vector.tensor_tensor_scan(q[:], av[:], bv[:], 0.0,
                                                 mybir.AluOpType.mult,
                                                 mybir.AluOpType.add)
                    lt = big.tile([128, TS], F32, tag="lt")
                    nc.vector.tensor_add(lt[:], q[:], qnc[:, j, 0:TS])
                    nc.sync.dma_start(lg_d[c, j], lt[:])


def _prep_core_inputs(actions, rewards, w_r1, b_r1, w_r2, b_r2, w_a1, b_a1,
                      w_a2, b_a2):
    """Host-side packing.  actions/rewards are the [B_LOC, T] core shard."""
    bf = ml_dtypes.bfloat16
    ts = TS

    w1 = np.zeros((70, 64), np.float32)
    w1[0:32, 0:32] = w_r1[:, 1:33].T
    w1[32:64, 32:64] = w_a1[:, 4:36].T
    w1[65, 0:32] = w_r1[:, 0]
    w1[66:70, 32:64] = w_a1[:, 0:4].T

    w2 = np.zeros((65, 8), np.float32)
    w2[32:64, 0:4] = w_a2.T
    w2[64, 0:4] = b_a2
    w2[0:32, 4:8] = w_r2[0][:, None]
    w2[64, 4:8] = b_r2[0]

    bias = np.concatenate([b_r1, b_a1]).astype(np.float32)[:, None]

    io = np.zeros((ts, 5, B_LOC), np.float32)
    io[:, 0, :] = rewards[:, :ts].T
    a_t = actions[:, :ts].T  # [ts, B_LOC]
    for k in range(N_ACT):
        io[:, 1 + k, :] = (a_t == k)

    init = np.zeros((65, B_LOC), np.float32)
    init[64] = 1.0

    abm = np.ascontiguousarray(
        actions[:, :ts].reshape(8, 128, ts)).astype(np.float32)

    return {
        "io": io,
        "w1": w1,
        "w2": w2,
        "bias": bias,
        "abm": abm,
        "eye": np.eye(128, dtype=np.float32),
        "init": init,
    }


def _get_nc():
    if "nc" not in _CACHE:
        _CACHE["nc"] = _build_nc()
    return _CACHE["nc"]


def run(inputs, **spmd_kwargs):
    """Build, run on 8 cores, return (output, BassKernelResults)."""
    nc = _get_nc()
    np_inputs = {k: np.asarray(v) for k, v in inputs.items()}
    actions = np_inputs["actions"]
    rewards = np_inputs["rewards"]
    params = {k: np_inputs[k] for k in
              ("w_r1", "b_r1", "w_r2", "b_r2", "w_a1", "b_a1", "w_a2", "b_a2")}

    in_maps = []
    for core in range(NCORES):
        sl = slice(core * B_LOC, (core + 1) * B_LOC)
        in_maps.append(
            _prep_core_inputs(actions[sl], rewards[sl], **params))

    res = run_bass_kernel_spmd(nc, in_maps, list(range(NCORES)),
                               **spmd_kwargs)

    out = np.empty((B, TS, N_ACT), np.float32)
    for core in range(NCORES):
        lg = res.results[core]["lg"]  # [8, 4, 128, TS]
        out[core * B_LOC:(core + 1) * B_LOC] = (
            lg.transpose(0, 2, 3, 1).reshape(B_LOC, TS, N_ACT))
    return out, res


def kernel(**inputs) -> np.ndarray:
    out, _ = run(inputs)
    return out


# revision 10
# speedup vs baseline: 1.2824x; 1.2824x over previous
"""Trainium2 Bass kernel for the MemoryANN recurrence.

Model (per batch row b, per step t over T-1 steps):
    sr_t = tanh(w_r1 @ [r_t; sr_{t-1}] + b_r1)          # 32-dim reward module
    qn_t = w_r2 @ sr_t + b_r2                           # scalar q_new
    sa_t = tanh(w_a1 @ [oh_t; sa_{t-1}] + b_a1)         # 32-dim action module
    c_t  = w_a2 @ sa_t + b_a2                           # 4-dim
    q_t  = 0.95*q_{t-1}*(1-oh_t) + qn_t*oh_t            # gated decay
    logits_t = q_t + c_t

Device strategy (pure data parallel, 1024 batch rows per core):
  Phase 1 (serial over t): feature-major state X [70, 1024] bf16 holds
  [sr(32); sa(32); ones(1); r(1); oh(4)].  One fused matmul produces both
  pre-tanh vectors (PSUM fp32), one ACT instr applies tanh+bias, a second
  fused matmul produces [c_t(4); qn_t(x4)] which is copied to SBUF and
  DMA'd to a DRAM scratch, feature-major.
  Phase 2 (batch-parallel): the q recurrence is a per-(b,j) linear scan
  q = A*q + Bv with A = 0.95*(1-oh), Bv = oh*qn.  Transpose qn/c to
  batch-major with PE transposes, then DVE tensor_tensor_scan runs the
  whole T recurrence in one instruction per (chunk, action).
"""

import sys

if "/root/.axon_site/_ro/trn_rl_repo" not in sys.path:  # pragma: no cover
    sys.path.insert(0, "/root/.axon_site/_ro/trn_rl_repo")

from contextlib import ExitStack

import numpy as np
import ml_dtypes

import concourse.bass as bass
import concourse.bacc as bacc
import concourse.tile as tile
from concourse import mybir
from concourse.bass_utils import run_bass_kernel_spmd

BF16 = mybir.dt.bfloat16
F32 = mybir.dt.float32
F32R = mybir.dt.float32r

import os
B, T = 8192, 1024
TS = int(os.environ.get("BASS_TS", T - 1))  # recurrence steps (default 1023)
NCORES = 8
B_LOC = B // NCORES  # 1024
N_ACT = 4
FORGET = 0.05
DECAY = 1.0 - FORGET

_CACHE = {}


def _build_nc():
    nc = bacc.Bacc("TRN2", target_bir_lowering=False, debug=False,
                   num_devices=NCORES)

    io_d = nc.dram_tensor("io", [TS, 5, B_LOC], F32, kind="ExternalInput")
    w1_d = nc.dram_tensor("w1", [70, 72], F32, kind="ExternalInput")
    bias_d = nc.dram_tensor("bias", [64, 1], F32, kind="ExternalInput")
    abm_d = nc.dram_tensor("abm", [8, 128, TS], F32, kind="ExternalInput")
    eye_d = nc.dram_tensor("eye", [128, 128], F32, kind="ExternalInput")
    init_d = nc.dram_tensor("init", [65, B_LOC], F32, kind="ExternalInput")
    out2_d = nc.dram_tensor("out2", [1024, 8, B_LOC], F32)  # internal scratch
    lg_d = nc.dram_tensor("lg", [8, N_ACT, 128, TS], F32, kind="ExternalOutput")

    with tile.TileContext(nc) as tc:
        _kernel(tc, io_d, w1_d, bias_d, abm_d, eye_d, init_d, out2_d, lg_d)
    nc.compile()
    return nc


def _kernel(tc, io_d, w1_d, bias_d, abm_d, eye_d, init_d, out2_d, lg_d):
    nc = tc.nc
    with ExitStack() as ctx:
        const = ctx.enter_context(tc.tile_pool(name="const", bufs=1))
        w1 = const.tile([70, 72], F32R, tag="w1")
        bias = const.tile([64, 1], F32, tag="bias")
        eye = const.tile([128, 128], F32, tag="eye")
        nc.sync.dma_start(w1[:], w1_d[:].bitcast(F32R))
        nc.sync.dma_start(bias[:], bias_d[:])
        nc.sync.dma_start(eye[:], eye_d[:])

        # ---------------- Phase 1: the serial recurrence ----------------
        # Two independent 512-column streams (A = cols 0:512, B = 512:1024).
        # MM2 for step t-1 is emitted after MM1/tanh of step t so the
        # in-order PE queue never stalls waiting for tanh(t).
        # Both MM2 outputs land in one PSUM bank (partitions 0-7 and
        # 32-39) so a single DVE copy drains them.
        with ExitStack() as p1ctx:
            xpool = p1ctx.enter_context(tc.tile_pool(name="xs", bufs=1))
            ps1 = p1ctx.enter_context(
                tc.tile_pool(name="ps1", bufs=3, space="PSUM"))
            ps2 = p1ctx.enter_context(
                tc.tile_pool(name="ps2", bufs=1, space="PSUM"))
            stag = p1ctx.enter_context(tc.tile_pool(name="stag", bufs=4))

            NX = 4  # ping-pong depth for the state/io buffer
            xs = [xpool.tile([70, B_LOC], F32R, tag=f"x{i}", name=f"x{i}")
                  for i in range(NX)]
            # init: zeros state for x0, ones row (bias multiplier) for all
            nc.sync.dma_start(xs[0][0:65, :], init_d[:].bitcast(F32R))
            for x in xs[1:]:
                nc.sync.dma_start(x[64:65, :], init_d[64:65, :].bitcast(F32R))

            # The merged matmul at iteration t produces, in one PSUM tile:
            #   rows 0-63:  pre-tanh for step t (from W_state/io columns)
            #   rows 64-71: [c; qn*4] of step t-1 (W2 columns read the
            #               sr/sa state rows, which hold step t-1's output)
            for t in range(TS):
                xc = xs[t % NX]
                xn = xs[(t + 1) % NX]
                nc.sync.dma_start(xc[65:70, :], io_d[t].bitcast(F32R))

                p1 = ps1.tile([72, B_LOC], F32, tag="p1")
                nc.tensor.matmul(p1[:, 0:512], w1[:], xc[:, 0:512])
                nc.tensor.matmul(p1[:, 512:1024], w1[:], xc[:, 512:1024])
                nc.scalar.activation(xn[0:64, 0:512], p1[0:64, 0:512],
                                     mybir.ActivationFunctionType.Tanh,
                                     bias=bias[:, 0:1])
                nc.scalar.activation(xn[0:64, 512:1024], p1[0:64, 512:1024],
                                     mybir.ActivationFunctionType.Tanh,
                                     bias=bias[:, 0:1])
                if t > 0:
                    st = stag.tile([8, B_LOC], F32, tag="st")
                    nc.vector.tensor_copy(st[:], p1[64:72, :])
                    nc.sync.dma_start(out2_d[t - 1], st[:])

            # epilogue: out2 of the last step, via the W2 columns only
            xq = xs[TS % NX]
            pe_ = ps2.tile([8, B_LOC], F32, tag="pe_")
            nc.tensor.matmul(pe_[:, 0:512], w1[:, 64:72], xq[:, 0:512])
            nc.tensor.matmul(pe_[:, 512:1024], w1[:, 64:72], xq[:, 512:1024])
            st = stag.tile([8, B_LOC], F32, tag="st")
            nc.vector.tensor_copy(st[:], pe_[:])
            nc.sync.dma_start(out2_d[TS - 1], st[:])

        # ---------------- Phase 2: batched q-scan ----------------
        with ExitStack() as p2ctx:
            pool = p2ctx.enter_context(tc.tile_pool(name="ph2", bufs=2))
            big = p2ctx.enter_context(tc.tile_pool(name="ph2big", bufs=2))
            pst = p2ctx.enter_context(
                tc.tile_pool(name="pst", bufs=2, space="PSUM"))

            for c in range(8):  # batch chunks of 128
                abm = big.tile([128, TS], F32, tag="abm")
                nc.sync.dma_start(abm[:], abm_d[c])
                qnc = big.tile([128, 5, 1024], F32, tag="qnc")
                for g in range(8):  # time groups of 128
                    fm = pool.tile([128, 5, 128], F32, tag="fm")
                    nc.sync.dma_start(
                        fm[:],
                        out2_d[g * 128:(g + 1) * 128, 0:5,
                               c * 128:(c + 1) * 128])
                    pt = pst.tile([128, 5, 128], F32, tag="pt")
                    for f in range(5):
                        nc.tensor.transpose(pt[:, f, :], fm[:, f, :], eye[:])
                    nc.vector.tensor_copy(
                        qnc[:, :, g * 128:(g + 1) * 128], pt[:])

                for j in range(N_ACT):
                    oh = big.tile([128, TS], F32, tag="oh")
                    nc.vector.tensor_scalar(oh[:], abm[:], float(j), None,
                                            mybir.AluOpType.is_equal)
                    av = big.tile([128, TS], F32, tag="av")
                    nc.vector.tensor_scalar(av[:], oh[:], -DECAY, DECAY,
                                            mybir.AluOpType.mult,
                                            mybir.AluOpType.add)
                    bv = big.tile([128, TS], F32, tag="bv")
                    nc.vector.tensor_tensor(bv[:], oh[:], qnc[:, 4, 0:TS],
                                            mybir.AluOpType.mult)
                    q = big.tile([128, TS], F32, tag="q")
                    nc.vector.tensor_tensor_scan(q[:], av[:], bv[:], 0.0,
                                                 mybir.AluOpType.mult,
                                                 mybir.AluOpType.add)
                    lt = big.tile([128, TS], F32, tag="lt")
                    nc.vector.tensor_add(lt[:], q[:], qnc[:, j, 0:TS])
                    nc.sync.dma_start(lg_d[c, j], lt[:])


def _prep_core_inputs(actions, rewards, w_r1, b_r1, w_r2, b_r2, w_a1, b_a1,
                      w_a2, b_a2):
    """Host-side packing.  actions/rewards are the [B_LOC, T] core shard."""
    bf = ml_dtypes.bfloat16
    ts = TS

    w1 = np.zeros((70, 72), np.float32)
    w1[0:32, 0:32] = w_r1[:, 1:33].T
    w1[32:64, 32:64] = w_a1[:, 4:36].T
    w1[65, 0:32] = w_r1[:, 0]
    w1[66:70, 32:64] = w_a1[:, 0:4].T
    # W2 columns: out rows 64-67 = c_t, 68-71 = qn (replicated x4)
    w1[32:64, 64:68] = w_a2.T
    w1[64, 64:68] = b_a2
    w1[0:32, 68:72] = w_r2[0][:, None]
    w1[64, 68:72] = b_r2[0]

    bias = np.concatenate([b_r1, b_a1]).astype(np.float32)[:, None]

    io = np.zeros((ts, 5, B_LOC), np.float32)
    io[:, 0, :] = rewards[:, :ts].T
    a_t = actions[:, :ts].T  # [ts, B_LOC]
    for k in range(N_ACT):
        io[:, 1 + k, :] = (a_t == k)

    init = np.zeros((65, B_LOC), np.float32)
    init[64] = 1.0

    abm = np.ascontiguousarray(
        actions[:, :ts].reshape(8, 128, ts)).astype(np.float32)

    return {
        "io": io,
        "w1": w1,
        "bias": bias,
        "abm": abm,
        "eye": np.eye(128, dtype=np.float32),
        "init": init,
    }


def _get_nc():
    if "nc" not in _CACHE:
        _CACHE["nc"] = _build_nc()
    return _CACHE["nc"]


def run(inputs, **spmd_kwargs):
    """Build, run on 8 cores, return (output, BassKernelResults)."""
    nc = _get_nc()
    np_inputs = {k: np.asarray(v) for k, v in inputs.items()}
    actions = np_inputs["actions"]
    rewards = np_inputs["rewards"]
    params = {k: np_inputs[k] for k in
              ("w_r1", "b_r1", "w_r2", "b_r2", "w_a1", "b_a1", "w_a2", "b_a2")}

    in_maps = []
    for core in range(NCORES):
        sl = slice(core * B_LOC, (core + 1) * B_LOC)
        in_maps.append(
            _prep_core_inputs(actions[sl], rewards[sl], **params))

    res = run_bass_kernel_spmd(nc, in_maps, list(range(NCORES)),
                               **spmd_kwargs)

    out = np.empty((B, TS, N_ACT), np.float32)
    for core in range(NCORES):
        lg = res.results[core]["lg"]  # [8, 4, 128, TS]
        out[core * B_LOC:(core + 1) * B_LOC] = (
            lg.transpose(0, 2, 3, 1).reshape(B_LOC, TS, N_ACT))
    return out, res


def kernel(**inputs) -> np.ndarray:
    out, _ = run(inputs)
    return out
